# revision 1
# baseline (speedup 1.0000x reference)
"""Trainium2 kernel for nn_DeQuPoolXYZ: out = E @ x @ E^H with
E = (V^H)^{otimes 6} a 4096x4096 unitary built from 6 rotation params.

Sharding: E-row-block per core.  Core c computes output rows
[512c, 512c+512): Y = E_c @ x (2 real matmuls, x real), then
Out_c^T = conj(E) @ Y^T computed directly in transposed orientation so
every matmul contracts over natural (row-major) partition chunks.

All matmuls use float32r (full-rate fp32 storage, ~1.6e-4 rel precision).
"""

import sys

for _p in ("/opt/trn_rl_repo",):
    if _p not in sys.path:
        sys.path.insert(0, _p)

import numpy as np

import concourse.bass as bass
import concourse.tile as tile
from concourse import mybir
from concourse.bass_utils import run_bass_kernel_spmd

DIM = 4096
P = 128
NCORES = 8
MROWS = DIM // NCORES      # 512 output rows per core
KC = DIM // P              # 32 chunks of 128

W_MUL = (2.0 ** 0.5) * (5.0 ** -0.5)

LAST_RESULTS = None        # BassKernelResults of the most recent run


# ---------------------------------------------------------------- host math
def _build_C(w):
    """64x64 unitary C = (V^H)^{otimes 3}; E = kron(C, C)."""
    w = np.asarray(w, np.float64) * W_MUL

    def rx(t):
        c, s = np.cos(t / 2), np.sin(t / 2)
        return np.array([[c, -1j * s], [-1j * s, c]])

    def ry(t):
        c, s = np.cos(t / 2), np.sin(t / 2)
        return np.array([[c, -s], [s, c]])

    def rz(t):
        e = np.exp(-0.5j * t)
        return np.array([[e, 0], [0, np.conj(e)]])

    I2 = np.eye(2)
    CNOT = np.array([[1, 0, 0, 0], [0, 1, 0, 0], [0, 0, 0, 1], [0, 0, 1, 0]],
                    dtype=complex)
    gates = [np.kron(rx(w[0]), I2), np.kron(I2, rx(w[1])),
             np.kron(ry(w[2]), I2), np.kron(I2, ry(w[3])),
             np.kron(rz(w[4]), I2), np.kron(I2, rz(w[5])), CNOT,
             np.kron(I2, rz(-w[5])), np.kron(I2, ry(-w[3])),
             np.kron(I2, rx(-w[1]))]
    V = np.eye(4, dtype=complex)
    for g in gates:
        V = g @ V
    Vh = V.conj().T
    return np.kron(np.kron(Vh, Vh), Vh)


# ---------------------------------------------------------------- device
def _legalize_waits(nc, keep=1, per_nop=1):
    """Move excess embedded sync waits onto same-engine NoOps (the walrus in
    this container caps waits/instruction at 1 for weight-load-class
    instructions and ~3 for others)."""
    counter = 0
    for fn in nc.m.functions:
        for blk in fn.blocks:
            insts = blk.instructions
            out = []
            changed = False
            for inst in insts:
                si = inst.sync_info
                waits = list(si.on_wait) if si and si.on_wait else []
                if len(waits) > keep:
                    excess = waits[:-keep] if keep else waits
                    kept = waits[-keep:] if keep else []
                    for i in range(0, len(excess), per_nop):
                        counter += 1
                        nop = mybir.InstNoOp(
                            name=f"I-WFIX-{counter}", ins=[], outs=[])
                        nop.engine = inst.engine
                        nop.sync_info = mybir.SyncInfo(
                            on_wait=excess[i:i + per_nop], on_update=[])
                        out.append(nop)
                    inst.sync_info = mybir.SyncInfo(
                        on_wait=kept,
                        on_update=list(si.on_update) if si.on_update else [])
                    changed = True
                out.append(inst)
            if changed:
                insts.clear()
                insts.extend(out)
    return counter


_NC_CACHE = None


def _build_bass():
    f32r = mybir.dt.float32r
    f32 = mybir.dt.float32
    nc = bass.Bass()

    x_d = nc.dram_tensor("x", [DIM, DIM], f32r, kind="ExternalInput")
    ect_r_d = nc.dram_tensor("ect_r", [DIM, MROWS], f32r, kind="ExternalInput")
    ect_i_d = nc.dram_tensor("ect_i", [DIM, MROWS], f32r, kind="ExternalInput")
    eh_r_d = nc.dram_tensor("eh_r", [DIM, DIM], f32r, kind="ExternalInput")
    eh_i_d = nc.dram_tensor("eh_i", [DIM, DIM], f32r, kind="ExternalInput")
    outt_r_d = nc.dram_tensor("outt_r", [DIM, MROWS], f32, kind="ExternalOutput")
    outt_i_d = nc.dram_tensor("outt_i", [DIM, MROWS], f32, kind="ExternalOutput")

    with tile.TileContext(nc) as tc:
        with tc.tile_pool(name="big", bufs=2) as big, \
             tc.tile_pool(name="dram", bufs=1, space="DRAM") as dram, \
             tc.tile_pool(name="mov", bufs=4) as mov, \
             tc.tile_pool(name="outs", bufs=3) as outs, \
             tc.tile_pool(name="ps1", bufs=2, space="PSUM") as ps1, \
             tc.tile_pool(name="ps2", bufs=1, space="PSUM") as ps2:

            yt_r_d = dram.tile([DIM, MROWS], f32r, tag="ytr")
            yt_i_d = dram.tile([DIM, MROWS], f32r, tag="yti")

            # ---------------- stage 1: YT = (E_c @ x)^T  (x real) ----------
            ect_r_sb = big.tile([P, KC, MROWS], f32r, tag="bigslot")
            nc.sync.dma_start(
                ect_r_sb, ect_r_d[:, :].rearrange("(ko p) m -> p ko m", p=P))
            ect_i_sb = big.tile([P, KC, MROWS], f32r, tag="bigslot")
            nc.sync.dma_start(
                ect_i_sb, ect_i_d[:, :].rearrange("(ko p) m -> p ko m", p=P))

            for n in range(KC):
                ps_r = ps1.tile([P, MROWS], f32, tag="psr")
                ps_i = ps1.tile([P, MROWS], f32, tag="psi")
                for k in range(KC):
                    xt = mov.tile([P, P], f32r, tag="xt")
                    nc.sync.dma_start(
                        xt, x_d[k * P:(k + 1) * P, n * P:(n + 1) * P])
                    nc.tensor.matmul(ps_r, xt, ect_r_sb[:, k, :],
                                     start=(k == 0), stop=(k == KC - 1))
                    nc.tensor.matmul(ps_i, xt, ect_i_sb[:, k, :],
                                     start=(k == 0), stop=(k == KC - 1))
                o_r = outs.tile([P, MROWS], f32r, tag="s1or")
                nc.vector.tensor_copy(o_r, ps_r)
                nc.sync.dma_start(yt_r_d[n * P:(n + 1) * P, :], o_r)
                o_i = outs.tile([P, MROWS], f32r, tag="s1oi")
                nc.vector.tensor_copy(o_i, ps_i)
                nc.sync.dma_start(yt_i_d[n * P:(n + 1) * P, :], o_i)

            # ---------------- stage 2: OutT = conj(E) @ YT -----------------
            yt_r_sb = big.tile([P, KC, MROWS], f32r, tag="bigslot")
            nc.sync.dma_start(
                yt_r_sb, yt_r_d[:, :].rearrange("(ko p) m -> p ko m", p=P))
            yt_i_sb = big.tile([P, KC, MROWS], f32r, tag="bigslot")
            nc.sync.dma_start(
                yt_i_sb, yt_i_d[:, :].rearrange("(ko p) m -> p ko m", p=P))

            for s in range(KC):
                b_rr = ps2.tile([P, MROWS], f32, tag="brr")
                b_ii = ps2.tile([P, MROWS], f32, tag="bii")
                b_im = ps2.tile([P, MROWS], f32, tag="bim")
                for n in range(KC):
                    ehr_t = mov.tile([P, P], f32r, tag="ehr")
                    nc.sync.dma_start(
                        ehr_t, eh_r_d[n * P:(n + 1) * P, s * P:(s + 1) * P])
                    ehi_t = mov.tile([P, P], f32r, tag="ehi")
                    nc.sync.dma_start(
                        ehi_t, eh_i_d[n * P:(n + 1) * P, s * P:(s + 1) * P])
                    first, last = (n == 0), (n == KC - 1)
                    nc.tensor.matmul(b_rr, ehr_t, yt_r_sb[:, n, :],
                                     start=first, stop=last)
                    nc.tensor.matmul(b_ii, ehi_t, yt_i_sb[:, n, :],
                                     start=first, stop=last)
                    nc.tensor.matmul(b_im, ehr_t, yt_i_sb[:, n, :],
                                     start=first, stop=False)
                    nc.tensor.matmul(b_im, ehi_t, yt_r_sb[:, n, :],
                                     start=False, stop=last)
                tmp = outs.tile([P, MROWS], f32, tag="s2tmp")
                nc.vector.tensor_copy(tmp, b_rr)
                o_r = outs.tile([P, MROWS], f32, tag="s2or")
                nc.vector.tensor_tensor(o_r, tmp, b_ii,
                                        mybir.AluOpType.subtract)
                nc.sync.dma_start(outt_r_d[s * P:(s + 1) * P, :], o_r)
                o_i = outs.tile([P, MROWS], f32, tag="s2oi")
                nc.vector.tensor_copy(o_i, b_im)
                nc.sync.dma_start(outt_i_d[s * P:(s + 1) * P, :], o_i)

    _legalize_waits(nc)
    return nc


def kernel(x, w):
    global LAST_RESULTS, _NC_CACHE
    C = _build_C(w)
    E = np.kron(C, C).astype(np.complex64)
    EH = np.ascontiguousarray(E.conj().T)
    eh_r = np.ascontiguousarray(EH.real.astype(np.float32))
    eh_i = np.ascontiguousarray(EH.imag.astype(np.float32))
    x32 = np.ascontiguousarray(np.asarray(x, dtype=np.float32))

    in_maps = []
    for c in range(NCORES):
        Ec = E[MROWS * c:MROWS * (c + 1), :]
        in_maps.append({
            "x": x32,
            "ect_r": np.ascontiguousarray(Ec.T.real.astype(np.float32)),
            "ect_i": np.ascontiguousarray(Ec.T.imag.astype(np.float32)),
            "eh_r": eh_r,
            "eh_i": eh_i,
        })

    if _NC_CACHE is None:
        _NC_CACHE = _build_bass()
    import os
    res = run_bass_kernel_spmd(
        _NC_CACHE, in_maps, core_ids=list(range(NCORES)),
        trace=bool(os.environ.get("BASS_TRACE")))
    LAST_RESULTS = res

    out = np.empty((DIM, DIM), np.complex64)
    for c, r in enumerate(res.results):
        out[MROWS * c:MROWS * (c + 1), :] = (
            r["outt_r"].T.astype(np.complex64)
            + np.complex64(1j) * r["outt_i"].T.astype(np.complex64))
    return out


# revision 2
# speedup vs baseline: 80794.0366x; 80794.0366x over previous
"""v2: Kronecker-factored kernel.  out = E x E^H, E = C (x) C.

Per core c (owns output rows [512c, 512c+512)):
  S1 contracts a' (sliced, strided x loads from HBM, col-tiled 4x),
  S2 contracts b' (full-complex K=128 via DRAM-bounced y1),
  T  PE-transposes y2 so s' lands on partitions,
  S3 contracts d' (block-diag over c'-parity),
  S4 contracts c' (full-complex K=128 via DRAM-bounced z).
All matmuls float32r.  Validated layout-exactly in v2sim.py.
"""

import sys

for _p in ("/opt/trn_rl_repo",):
    if _p not in sys.path:
        sys.path.insert(0, _p)

import numpy as np

import concourse.bass as bass
import concourse.tile as tile
from concourse import mybir
from concourse.bass_utils import run_bass_kernel_spmd

DIM = 4096
P = 128
NCORES = 8
MROWS = 512

W_MUL = (2.0 ** 0.5) * (5.0 ** -0.5)
LAST_RESULTS = None


def _build_C(w):
    w = np.asarray(w, np.float64) * W_MUL

    def rx(t):
        c, s = np.cos(t / 2), np.sin(t / 2)
        return np.array([[c, -1j * s], [-1j * s, c]])

    def ry(t):
        c, s = np.cos(t / 2), np.sin(t / 2)
        return np.array([[c, -s], [s, c]])

    def rz(t):
        e = np.exp(-0.5j * t)
        return np.array([[e, 0], [0, np.conj(e)]])

    I2 = np.eye(2)
    CNOT = np.array([[1, 0, 0, 0], [0, 1, 0, 0], [0, 0, 0, 1], [0, 0, 1, 0]],
                    dtype=complex)
    gates = [np.kron(rx(w[0]), I2), np.kron(I2, rx(w[1])),
             np.kron(ry(w[2]), I2), np.kron(I2, ry(w[3])),
             np.kron(rz(w[4]), I2), np.kron(I2, rz(w[5])), CNOT,
             np.kron(I2, rz(-w[5])), np.kron(I2, ry(-w[3])),
             np.kron(I2, rx(-w[1]))]
    V = np.eye(4, dtype=complex)
    for g in gates:
        V = g @ V
    Vh = V.conj().T
    return np.kron(np.kron(Vh, Vh), Vh)


def _host_weights(C, c):
    """Device weight matrices for core c (see v2sim.py)."""
    Cc = C[8 * c:8 * c + 8, :]
    Cr, Ci = C.real.astype(np.float32), C.imag.astype(np.float32)
    Ccr, Cci = Cc.real.astype(np.float32), Cc.imag.astype(np.float32)

    CA = np.zeros((64, 16), np.float32)
    for ri in range(2):
        for ah in range(8):
            CA[:, 8 * ri + ah] = (Ccr if ri == 0 else Cci)[ah, :]
    W1 = np.zeros((128, 32), np.float32)
    W1[:64, :16] = CA
    W1[64:, 16:] = CA

    CB = np.zeros((128, 128), np.float32)
    CB[:64, :64] = Cr.T
    CB[64:, :64] = -Ci.T
    CB[:64, 64:] = Ci.T
    CB[64:, 64:] = Cr.T

    def B3(ri_in, h):
        B = np.zeros((64, 64), np.float32)
        dl = slice(32 * h, 32 * h + 32)
        if ri_in == 0:
            B[:, 0::2] = Cr[dl, :].T
            B[:, 1::2] = -Ci[dl, :].T
        else:
            B[:, 0::2] = Ci[dl, :].T
            B[:, 1::2] = Cr[dl, :].T
        W = np.zeros((128, 128), np.float32)
        W[:64, :64] = B
        W[64:, 64:] = B
        return W

    W3 = np.stack([B3(ri, h) for ri in range(2) for h in range(2)])  # [4,128,128]

    CD = np.zeros((128, 128), np.float32)
    CD[:64, :64] = Cr.T
    CD[64:, :64] = Ci.T
    CD[:64, 64:] = -Ci.T
    CD[64:, 64:] = Cr.T
    return W1, CB, W3, CD


def _legalize_waits(nc, keep=1, per_nop=1):
    counter = 0
    for fn in nc.m.functions:
        for blk in fn.blocks:
            insts = blk.instructions
            out = []
            changed = False
            for inst in insts:
                si = inst.sync_info
                waits = list(si.on_wait) if si and si.on_wait else []
                if len(waits) > keep:
                    excess = waits[:-keep] if keep else waits
                    kept = waits[-keep:] if keep else []
                    for i in range(0, len(excess), per_nop):
                        counter += 1
                        nop = mybir.InstNoOp(
                            name=f"I-WFIX-{counter}", ins=[], outs=[])
                        nop.engine = inst.engine
                        nop.sync_info = mybir.SyncInfo(
                            on_wait=excess[i:i + per_nop], on_update=[])
                        out.append(nop)
                    inst.sync_info = mybir.SyncInfo(
                        on_wait=kept,
                        on_update=list(si.on_update) if si.on_update else [])
                    changed = True
                out.append(inst)
            if changed:
                insts.clear()
                insts.extend(out)
    return counter


_NC_CACHE = None

# dtype knobs: "f32r" or "fp16" (experiments; kernel ships with the
# combination validated against the reference)
X_DT = "fp16"      # x + W1 (stage S1)
B_DT = "f32r"      # y1/z bounces + CB/W3/CD + downstream matmuls


def _build_bass():
    f32r = mybir.dt.float32r
    f32 = mybir.dt.float32
    xdt = {"f32r": mybir.dt.float32r, "fp16": mybir.dt.float16}[X_DT]
    bdt = {"f32r": mybir.dt.float32r, "fp16": mybir.dt.float16}[B_DT]
    nc = bass.Bass()

    x_d = nc.dram_tensor("x", [DIM, DIM], xdt, kind="ExternalInput")
    w1_d = nc.dram_tensor("w1", [128, 32], xdt, kind="ExternalInput")
    cb_d = nc.dram_tensor("cb", [128, 128], bdt, kind="ExternalInput")
    w3_d = nc.dram_tensor("w3", [4, 128, 128], bdt, kind="ExternalInput")
    cd_d = nc.dram_tensor("cd", [128, 128], bdt, kind="ExternalInput")
    id_d = nc.dram_tensor("ident", [128, 128], bdt, kind="ExternalInput")
    outt_r_d = nc.dram_tensor("outt_r", [DIM, MROWS], f32, kind="ExternalOutput")
    outt_i_d = nc.dram_tensor("outt_i", [DIM, MROWS], f32, kind="ExternalOutput")

    # x viewed as [b', a', s] so partition dim = a' (stride 64 rows)
    x_bav = x_d[:, :].rearrange("(a b) s -> b a s", b=64)
    # outt viewed as [delta, gamma, m] (rows s = 64*gamma + delta)
    outr_v = outt_r_d[:, :].rearrange("(g d) m -> d g m", d=64)
    outi_v = outt_i_d[:, :].rearrange("(g d) m -> d g m", d=64)

    with tile.TileContext(nc) as tc:
        with tc.tile_pool(name="wts", bufs=1) as wts, \
             tc.tile_pool(name="mov", bufs=5) as mov, \
             tc.tile_pool(name="xmv", bufs=2) as xmv, \
             tc.tile_pool(name="stage", bufs=2) as stage, \
             tc.tile_pool(name="big", bufs=1) as big, \
             tc.tile_pool(name="outs", bufs=2) as outs, \
             tc.tile_pool(name="dram", bufs=1, space="DRAM") as dram, \
             tc.tile_pool(name="psA", bufs=2, space="PSUM") as psA, \
             tc.tile_pool(name="psB", bufs=2, space="PSUM") as psB, \
             tc.tile_pool(name="psT", bufs=2, space="PSUM") as psT, \
             tc.tile_pool(name="ps3", bufs=2, space="PSUM") as ps3:

            w1_sb = wts.tile([128, 32], xdt, tag="w1")
            nc.sync.dma_start(w1_sb, w1_d[:, :])
            cb_sb = wts.tile([128, 128], bdt, tag="cb")
            nc.sync.dma_start(cb_sb, cb_d[:, :])
            w3_sb = wts.tile([128, 4, 128], bdt, tag="w3")
            nc.sync.dma_start(w3_sb, w3_d[:, :, :].rearrange("k p m -> p k m"))
            cd_sb = wts.tile([128, 128], bdt, tag="cd")
            nc.sync.dma_start(cd_sb, cd_d[:, :])
            ident = wts.tile([128, 128], bdt, tag="ident")
            nc.sync.dma_start(ident, id_d[:, :])

            y1th = []
            for i in range(2):
                y1half = dram.tile([1024, DIM // 2], bdt, tag=f"y1h{i}",
                                   name=f"y1h{i}")
                y1th.append(y1half)
            # y1 rows = 128*ah + 64*ri + 8*o + 2*j + z,  o = 4*oh + oo
            y1_wh = [y1th[i][:, :].rearrange(
                "(ah ri oh oo j z) s -> ri z j oh ah oo s",
                ah=8, ri=2, oh=2, oo=4, j=4, z=2) for i in range(2)]

            # ------------------------- S1 -------------------------
            # col-tiled: psum partitions (j, z, ri, ah) = 32j + 16z + 8ri + ah
            for ts in range(2):
                for oh in range(2):
                    stg = stage.tile([128, 4, 2048], bdt, tag="y1stage")
                    for oo in range(4):
                        o = 4 * oh + oo
                        mvs = []
                        for j in range(4):
                            mv = xmv.tile([128, 2048], xdt, tag=f"xmov{j}")
                            for z in range(2):
                                bp = 8 * o + 2 * j + z
                                nc.sync.dma_start(
                                    mv[64 * z:64 * z + 64, :],
                                    x_bav[bp, :, 2048 * ts:2048 * ts + 2048])
                            mvs.append(mv)
                        for tl in range(4):
                            ps = psA.tile([128, 512], f32, tag="psA")
                            for j in range(4):
                                nc.tensor.matmul(
                                    ps[32 * j:32 * j + 32, :], w1_sb,
                                    mvs[j][:, 512 * tl:512 * tl + 512],
                                    start=True, stop=True,
                                    tile_position=(0, 32 * j))
                            nc.vector.tensor_copy(
                                stg[:, oo, 512 * tl:512 * tl + 512], ps)
                    for j in range(4):
                        for z in range(2):
                            for ri in range(2):
                                pbase = 32 * j + 16 * z + 8 * ri
                                nc.sync.dma_start(
                                    y1_wh[ts][ri, z, j, oh, :, :, :],
                                    stg[pbase:pbase + 8, :, :])

            # -------------------- S2 + T + S3 (quarters) ----------
            zt = dram.tile([8192, 512], bdt, tag="z")
            # z rows = 4096*h + 64*(2*dh+rii) + 2*kh + w
            z_v = zt[:, :].rearrange(
                "(h p cp) m -> p cp h m", h=2, p=64, cp=64)
            for q in range(4):
                y2T = big.tile([128, 8, 8, 128], bdt, tag="y2T")
                for tq in range(2):
                    t = 2 * q + tq
                    for ah in range(8):
                        mv = mov.tile([128, 512], bdt, tag="y2mov")
                        nc.sync.dma_start(
                            mv, y1th[t // 4][128 * ah:128 * ah + 128,
                                             512 * (t % 4):512 * (t % 4) + 512])
                        ps2 = psB.tile([128, 512], f32, tag="psB")
                        nc.tensor.matmul(ps2, cb_sb, mv, start=True, stop=True)
                        y2s = stage.tile([128, 512], bdt, tag="y2sb")
                        nc.vector.tensor_copy(y2s, ps2)
                        for u in range(4):
                            pst = psT.tile([128, 128], bdt, tag="psT")
                            nc.tensor.transpose(
                                pst, y2s[:, 128 * u:128 * u + 128], ident)
                            kloc = 4 * tq + u
                            nc.scalar.copy(y2T[:, kloc, ah, :], pst)
                for kl in range(8):
                    kh = 8 * q + kl
                    z_s = outs.tile([128, 2, 512], bdt, tag="zstg")
                    for h in range(2):
                        p3 = ps3.tile([128, 512], f32, tag="ps3")
                        for ri_in in range(2):
                            mv_ap = y2T[:, kl, :, 64 * ri_in:64 * ri_in + 64]
                            nc.tensor.matmul(p3, w3_sb[:, 2 * ri_in + h, :],
                                             mv_ap, start=(ri_in == 0),
                                             stop=(ri_in == 1))
                        nc.vector.tensor_copy(z_s[:, h, :], p3)
                    for w in range(2):
                        nc.sync.dma_start(
                            z_v[:, 2 * kh + w, :, :],
                            z_s[64 * w:64 * w + 64, :, :])

            # ------------------------- S4 -------------------------
            outr_q = outt_r_d[:, :].rearrange("(g d) m -> g d m", d=64)
            outi_q = outt_i_d[:, :].rearrange("(g d) m -> g d m", d=64)
            for dg in range(16):
                o_s = outs.tile([128, 4, 512], f32, tag="ostg")
                for dq in range(4):
                    delta = 4 * dg + dq
                    mv = mov.tile([128, 512], bdt, tag="zmov")
                    nc.sync.dma_start(
                        mv, zt[128 * delta:128 * delta + 128, :])
                    ps4 = psB.tile([128, 512], f32, tag="psB")
                    nc.tensor.matmul(ps4, cd_sb, mv, start=True, stop=True)
                    nc.vector.tensor_copy(o_s[:, dq, :], ps4)
                nc.sync.dma_start(outr_q[:, 4 * dg:4 * dg + 4, :],
                                  o_s[0:64, :, :])
                nc.sync.dma_start(outi_q[:, 4 * dg:4 * dg + 4, :],
                                  o_s[64:128, :, :])

    _legalize_waits(nc)
    return nc


def kernel(x, w):
    global LAST_RESULTS, _NC_CACHE
    C = _build_C(w).astype(np.complex64)
    x32 = np.ascontiguousarray(np.asarray(x, dtype=np.float32))

    xnp = {"f32r": np.float32, "fp16": np.float16}[X_DT]
    bnp = {"f32r": np.float32, "fp16": np.float16}[B_DT]
    x_cast = np.ascontiguousarray(x32.astype(xnp))
    in_maps = []
    shared = None
    for c in range(NCORES):
        W1, CB, W3, CD = _host_weights(C, c)
        if shared is None:
            shared = (CB.astype(bnp), W3.astype(bnp), CD.astype(bnp))
        in_maps.append({
            "x": x_cast,
            "w1": W1.astype(xnp),
            "cb": shared[0],
            "w3": shared[1],
            "cd": shared[2],
            "ident": np.eye(128, dtype=bnp),
        })

    if _NC_CACHE is None:
        _NC_CACHE = _build_bass()
    import os
    res = run_bass_kernel_spmd(
        _NC_CACHE, in_maps, core_ids=list(range(NCORES)),
        trace=bool(os.environ.get("BASS_TRACE")))
    LAST_RESULTS = res

    out = np.empty((DIM, DIM), np.complex64)
    for c, r in enumerate(res.results):
        out[MROWS * c:MROWS * (c + 1), :] = (
            r["outt_r"].T.astype(np.complex64)
            + np.complex64(1j) * r["outt_i"].T.astype(np.complex64))
    return out
